# revision 23
# baseline (speedup 1.0000x reference)
"""DGI (2-layer GCN encoder) Trainium2 kernel, 8-core SPMD.

Strategy:
- Fuse positive/negative streams as 128-wide features: row c of the launch-1
  gather table = [x[c], x[perm[c]]] * dis[c]  (512B rows).
- Shard destination nodes across 8 cores (12500 rows each, padded to 98
  blocks of 128 rows, processed in superblocks of 8 blocks). Each core
  bulk-gathers source rows from its local HBM copy of the table with
  dma_gather (int16 indices -> 4 table chunks of 32768 rows), segment-sums
  them via one-hot S-matrix matmuls accumulating into PSUM (aggT[fi, r]),
  then applies the 128x128 block-diagonal weight once per 128-row block,
  dis[r] as a per-partition scale on the PSUM->SBUF copy, bias add, PReLU.
- Between layers the host reassembles the full 100k x 128 activation table
  (launch-1 output rows are already dis[r]-scaled = exactly the layer-2
  gather table) and restages it; launch 2 repeats the pipeline with W2/b2,
  no PReLU, and emits masked per-feature column sums for the summary.
"""
import sys
import numpy as np

sys.path.insert(0, "/opt/trn_rl_repo")

import concourse.bass as bass
import concourse.bacc as bacc
import concourse.mybir as mybir
import concourse.tile as tile
from concourse.bass_utils import run_bass_kernel_spmd

N_NODES = 100000
N_CORES = 8
RPC = N_NODES // N_CORES          # 12500 destination rows per core
NB = (RPC + 127) // 128           # 98 blocks of 128 rows
BR = NB * 128                     # 12544 padded rows per core
F2 = 128                          # fused feature width (pos|neg)
SBB = 8                           # blocks per superblock
NSB = (NB + SBB - 1) // SBB       # 13 superblocks
# per-core rotated table: core p stages rows rotated by (p*RPC - ROT0) so its
# own 12544 rows sit inside chunk 0 for every core (uniform schedule).
ROT0 = 4096
CHUNK_BOUNDS = [0, 24576, 49718, 74859, 100000]   # widths < 32768
NCHUNK = len(CHUNK_BOUNDS) - 1
PRELU_A = 0.25
PAD_DST = 200.0                   # dst sentinel -> no one-hot match in S


def _build_launch(sched, final, prelu_a=PRELU_A, repeat=1):
    """One SPMD launch over NB blocks.

    sched: dict with the shared compile-time tile schedule:
      T_bc [NB, NCHUNK] tiles per (block, chunk); ni_sc [NSB][NCHUNK]
      gather sizes; NT total tiles; NIDX total index slots.
    final=False: layer 1 -> writes dis[r]*prelu(conv1) rows (layer-2 table).
    final=True:  layer 2 -> writes conv2 rows + masked column sums (summary).
    """
    T_bc, ni_sc, NT, NIDX = sched["T_bc"], sched["ni_sc"], sched["NT"], sched["NIDX"]
    nc = bacc.Bacc(num_swdge_queues=4)
    f32 = mybir.dt.float32
    d_table = nc.declare_dram_parameter("table", [N_NODES, F2], f32, isOutput=False)
    d_idx = nc.declare_dram_parameter("idx", [128, NIDX // 16], mybir.dt.int16, isOutput=False)
    d_dst = nc.declare_dram_parameter("dst", [128, NT], f32, isOutput=False)
    d_iota = nc.declare_dram_parameter("iota", [128, 128], f32, isOutput=False)
    d_w = nc.declare_dram_parameter("w", [F2, F2], f32, isOutput=False)
    d_bb = nc.declare_dram_parameter("bb", [128, 4 * F2], f32, isOutput=False)
    d_dis = nc.declare_dram_parameter("dis", [128, NB], f32, isOutput=False)
    d_out = nc.declare_dram_parameter("out", [BR, F2], f32, isOutput=True)
    if final:
        d_mask = nc.declare_dram_parameter("mask", [128, NB], f32, isOutput=False)
        d_sum = nc.declare_dram_parameter("colsum", [128, 1], f32, isOutput=True)

    GRP = 4  # blocks per epilogue group
    with tile.TileContext(nc) as tc:
        with (
            tc.tile_pool(name="const", bufs=1) as cpool,
            tc.tile_pool(name="gat", bufs=3) as gpool,
            tc.tile_pool(name="sbl", bufs=2) as spool,
            tc.tile_pool(name="agc", bufs=2) as apool,
            tc.tile_pool(name="epi", bufs=3) as epool,
            tc.tile_pool(name="pa", bufs=2, space="PSUM") as pa,
            tc.tile_pool(name="py", bufs=2, space="PSUM") as py,
            tc.tile_pool(name="ps", bufs=1, space="PSUM") as ps,
        ):
            t_idx = cpool.tile([128, NIDX // 16], mybir.dt.int16)
            t_dst = cpool.tile([128, NT], f32)
            t_iota = cpool.tile([128, 128], f32)
            t_w = cpool.tile([F2, F2], f32)
            t_bb = cpool.tile([128, 4 * F2], f32)
            t_dis = cpool.tile([128, NB], f32)
            nc.sync.dma_start(out=t_idx[:], in_=d_idx[:])
            nc.sync.dma_start(out=t_dst[:], in_=d_dst[:])
            nc.sync.dma_start(out=t_iota[:], in_=d_iota[:])
            nc.sync.dma_start(out=t_w[:], in_=d_w[:])
            nc.sync.dma_start(out=t_bb[:], in_=d_bb[:])
            nc.sync.dma_start(out=t_dis[:], in_=d_dis[:])
            if final:
                t_mask = cpool.tile([128, NB], f32)
                nc.sync.dma_start(out=t_mask[:], in_=d_mask[:])
                t_sumps = ps.tile([128, 1], f32, space="PSUM")

            def _one_pass():
              idx_off = 0   # in 16-wide slots
              tile_off = 0  # global tile counter
              for sb in range(NSB):
                blks = list(range(sb * SBB, min((sb + 1) * SBB, NB)))
                nblk = len(blks)
                aggT = pa.tile([128, SBB * 128], f32, space="PSUM", tag="aggT")
                # emission order of aggT matmuls; start/stop once per 2KB
                # PSUM zero-region (4 block-slices of 128 fp32 columns)
                seq = [(c, j, t)
                       for c in range(NCHUNK)
                       for j, b in enumerate(blks)
                       for t in range(int(T_bc[b, c]))]
                region = lambda j: j // 4
                first_of = {}
                last_of = {}
                for k, (c, j, t) in enumerate(seq):
                    r = region(j)
                    if r not in first_of:
                        first_of[r] = k
                    last_of[r] = k
                k_iter = 0
                for c in range(NCHUNK):
                    ni = ni_sc[sb][c]
                    if ni == 0:
                        continue
                    ntile = ni // 128
                    gbuf = gpool.tile([128, ntile * F2], f32, tag="gbuf")
                    nc.gpsimd.dma_gather(
                        out_ap=gbuf[:].rearrange("p (t f) -> p t f", f=F2),
                        in_ap=d_table[CHUNK_BOUNDS[c]:CHUNK_BOUNDS[c + 1], :],
                        idxs_ap=t_idx[:, idx_off:idx_off + ni // 16],
                        num_idxs=ni, num_idxs_reg=ni,
                        elem_size=F2, single_packet=False,
                        queue_num=(sb * NCHUNK + c) % 4,
                    )
                    idx_off += ni // 16
                    S = spool.tile([128, ntile * 128], f32, tag="S")
                    nc.vector.tensor_tensor(
                        out=S[:].rearrange("p (t r) -> p t r", t=ntile),
                        in0=t_dst[:, tile_off:tile_off + ntile]
                            .to_broadcast([128, ntile, 128]),
                        in1=t_iota[:, :].rearrange("p (o r) -> p o r", o=1)
                            .to_broadcast([128, ntile, 128]),
                        op=mybir.AluOpType.is_equal,
                    )
                    ti = 0
                    for j, b in enumerate(blks):
                        Tb = int(T_bc[b, c])
                        for t in range(Tb):
                            r = region(j)
                            nc.tensor.matmul(
                                out=aggT[:, j * 128:(j + 1) * 128],
                                lhsT=gbuf[:, ti * F2:(ti + 1) * F2],
                                rhs=S[:, ti * 128:(ti + 1) * 128],
                                start=(first_of[r] == k_iter),
                                stop=(last_of[r] == k_iter),
                                skip_group_check=True,
                            )
                            ti += 1
                            k_iter += 1
                    assert ti == ntile
                    tile_off += ntile
                aggTs = apool.tile([128, SBB * 128], f32, tag="aggTs")
                nc.vector.tensor_copy(out=aggTs[:, :nblk * 128],
                                      in_=aggT[:, :nblk * 128])
                for g0 in range(0, nblk, GRP):
                    gblks = blks[g0:g0 + GRP]
                    ng = len(gblks)
                    y_grp = py.tile([128, GRP * F2], f32, space="PSUM", tag="y")
                    o_grp = epool.tile([128, GRP * F2], f32, tag="o")
                    for j, b in enumerate(gblks):
                        nc.tensor.matmul(
                            out=y_grp[:, j * F2:(j + 1) * F2],
                            lhsT=aggTs[:, (g0 + j) * 128:(g0 + j + 1) * 128],
                            rhs=t_w[:], start=True, stop=True)
                        nc.scalar.activation(
                            out=o_grp[:, j * F2:(j + 1) * F2],
                            in_=y_grp[:, j * F2:(j + 1) * F2],
                            func=mybir.ActivationFunctionType.Copy,
                            scale=t_dis[:, b:b + 1])
                    w_grp = slice(0, ng * F2)
                    nc.vector.tensor_tensor(out=o_grp[:, w_grp], in0=o_grp[:, w_grp],
                                            in1=t_bb[:, w_grp], op=mybir.AluOpType.add)
                    if not final:
                        nc.vector.scalar_tensor_tensor(
                            out=o_grp[:, w_grp], in0=o_grp[:, w_grp],
                            scalar=float(prelu_a), in1=o_grp[:, w_grp],
                            op0=mybir.AluOpType.mult, op1=mybir.AluOpType.max)
                        for j, b in enumerate(gblks):
                            nc.scalar.activation(
                                out=o_grp[:, j * F2:(j + 1) * F2],
                                in_=o_grp[:, j * F2:(j + 1) * F2],
                                func=mybir.ActivationFunctionType.Copy,
                                scale=t_dis[:, b:b + 1])
                    else:
                        for j, b in enumerate(gblks):
                            nc.tensor.matmul(
                                out=t_sumps[:],
                                lhsT=o_grp[:, j * F2:(j + 1) * F2],
                                rhs=t_mask[:, b:b + 1],
                                start=(b == 0), stop=(b == NB - 1))
                    nc.sync.dma_start(
                        out=d_out[gblks[0] * 128:(gblks[0] + ng) * 128, :]
                            .rearrange("(b p) f -> p b f", p=128),
                        in_=o_grp[:, w_grp].rearrange("p (b f) -> p b f", f=F2),
                    )
            for _rep in range(repeat):
                _one_pass()
            if final:
                sums = epool.tile([128, 1], f32, tag="sums")
                nc.vector.tensor_copy(out=sums[:], in_=t_sumps[:])
                nc.sync.dma_start(out=d_sum[:], in_=sums[:])
    nc.finalize()
    return nc


def _prep_graph(edge_index):
    """Host-side index prep. Returns shared schedule + per-core arrays."""
    ei = np.asarray(edge_index)
    src = ei[0].astype(np.int64)
    dstc = ei[1].astype(np.int64)
    loops = np.arange(N_NODES, dtype=np.int64)
    rows = np.concatenate([src, loops])       # destination of each message
    cols = np.concatenate([dstc, loops])      # source of each message
    deg = np.bincount(rows, minlength=N_NODES).astype(np.float32)
    dis = (1.0 / np.sqrt(deg)).astype(np.float32)

    bounds = np.asarray(CHUNK_BOUNDS)
    cores = []
    counts_all = np.zeros((N_CORES, NB, NCHUNK), np.int64)
    for p in range(N_CORES):
        lo = p * RPC
        m = (rows >= lo) & (rows < lo + RPC)
        r_loc = rows[m] - lo
        # rotated source id: core table row = (col - (lo - ROT0)) mod N
        c_loc = (cols[m] - (lo - ROT0)) % N_NODES
        blk = r_loc >> 7
        chunk = np.searchsorted(bounds, c_loc, side="right") - 1
        order = np.lexsort((c_loc, chunk, blk))
        r_loc, c_loc, blk, chunk = (a[order] for a in (r_loc, c_loc, blk, chunk))
        key = blk * NCHUNK + chunk
        counts_all[p] = np.bincount(key, minlength=NB * NCHUNK).reshape(NB, NCHUNK)
        cores.append((r_loc, c_loc, key))

    T_bc = (counts_all.max(axis=0) + 127) >> 7       # [NB, NCHUNK] shared
    # gather sizes per (superblock, chunk)
    ni_sc = [[int(T_bc[sb * SBB:(sb + 1) * SBB, c].sum()) * 128
              for c in range(NCHUNK)] for c_ in [0] for sb in range(NSB)]
    NT = int(T_bc.sum())
    NIDX = NT * 128

    # per-(b,c) slot offsets in schedule order (sb, c, b, t)
    slot_off = np.zeros((NB, NCHUNK), np.int64)
    off = 0
    order_bc = []
    for sb in range(NSB):
        for c in range(NCHUNK):
            for b in range(sb * SBB, min((sb + 1) * SBB, NB)):
                slot_off[b, c] = off
                off += int(T_bc[b, c])
                order_bc.append((b, c))
    assert off == NT

    idx_all, dst_all, dis_all, mask_all = [], [], [], []
    for p in range(N_CORES):
        r_loc, c_loc, key = cores[p]
        starts = np.zeros(NB * NCHUNK + 1, np.int64)
        np.cumsum(np.bincount(key, minlength=NB * NCHUNK), out=starts[1:])
        idx_flat = np.zeros(NT * 128, np.int64)
        dst_flat = np.full(NT * 128, PAD_DST, np.float32)
        for b in range(NB):
            for c in range(NCHUNK):
                nE = int(counts_all[p, b, c])
                if T_bc[b, c] == 0:
                    continue
                s0 = int(slot_off[b, c]) * 128
                cap = int(T_bc[b, c]) * 128
                sl = slice(starts[b * NCHUNK + c], starts[b * NCHUNK + c] + nE)
                idx_flat[s0:s0 + nE] = c_loc[sl] - CHUNK_BOUNDS[c]
                if nE:
                    idx_flat[s0 + nE:s0 + cap] = idx_flat[s0 + nE - 1]
                dst_flat[s0:s0 + nE] = (r_loc[sl] & 127).astype(np.float32)
        # idx16: index j of the flat stream at [16*(rep) + j%16, j//16]
        idx16 = np.zeros((128, NT * 8), np.int16)
        w = idx_flat.reshape(NT * 8, 16).T.astype(np.int16)
        for rep in range(8):
            idx16[rep * 16:(rep + 1) * 16, :] = w
        # dst: tile t column, partition e = edge (t*128+e)
        dst_dev = np.ascontiguousarray(dst_flat.reshape(NT, 128).T)
        dis_pad = np.zeros(BR, np.float32)
        dis_pad[:RPC] = dis[p * RPC:(p + 1) * RPC]
        dis_dev = np.ascontiguousarray(dis_pad.reshape(NB, 128).T)
        maskv = np.zeros(BR, np.float32)
        maskv[:RPC] = 1.0
        mask_dev = np.ascontiguousarray(maskv.reshape(NB, 128).T)
        idx_all.append(idx16); dst_all.append(dst_dev)
        dis_all.append(dis_dev); mask_all.append(mask_dev)

    sched = {"T_bc": T_bc, "ni_sc": ni_sc, "NT": NT, "NIDX": NIDX}
    return sched, dis, idx_all, dst_all, dis_all, mask_all


_iota = np.tile(np.arange(128, dtype=np.float32), (128, 1))


def _launch(nc, base_maps, extra):
    in_maps = [dict(bm, **ex) for bm, ex in zip(base_maps, extra)]
    return run_bass_kernel_spmd(nc, in_maps, list(range(N_CORES))).results


def _bd(W):
    Z = np.zeros((F2, F2), np.float32)
    Z[:64, :64] = W
    Z[64:, 64:] = W
    return Z


def kernel(x, edge_index, perm, W1, b1, prelu_a, W2, b2):
    x = np.asarray(x, np.float32)
    W1 = np.asarray(W1, np.float32); b1 = np.asarray(b1, np.float32)
    W2 = np.asarray(W2, np.float32); b2 = np.asarray(b2, np.float32)
    perm_np = np.asarray(perm).astype(np.int64)

    sched, dis, idx_all, dst_all, dis_all, mask_all = _prep_graph(edge_index)

    nc1 = _build_launch(sched, final=False, prelu_a=float(np.asarray(prelu_a)))
    nc2 = _build_launch(sched, final=True)

    table1 = np.concatenate([x, x[perm_np]], axis=1) * dis[:, None]
    bb1 = np.tile(np.concatenate([b1, b1])[None, :], (128, 4)).astype(np.float32)
    bb2 = np.tile(np.concatenate([b2, b2])[None, :], (128, 4)).astype(np.float32)

    def rot(tab, p):
        s = (p * RPC - ROT0) % N_NODES
        return np.ascontiguousarray(np.concatenate([tab[s:], tab[:s]], axis=0))

    base1 = [{"table": rot(table1, p), "iota": _iota, "w": _bd(W1), "bb": bb1}
             for p in range(N_CORES)]
    extra1 = [{"idx": idx_all[p], "dst": dst_all[p], "dis": dis_all[p]}
              for p in range(N_CORES)]
    res1 = _launch(nc1, base1, extra1)

    table2 = np.empty((N_NODES, F2), np.float32)
    for p in range(N_CORES):
        table2[p * RPC:(p + 1) * RPC] = res1[p]["out"][:RPC]

    base2 = [{"table": rot(table2, p), "iota": _iota, "w": _bd(W2), "bb": bb2}
             for p in range(N_CORES)]
    extra2 = [{"idx": idx_all[p], "dst": dst_all[p], "dis": dis_all[p],
               "mask": mask_all[p]} for p in range(N_CORES)]
    res2 = _launch(nc2, base2, extra2)

    positive = np.empty((N_NODES, 64), np.float32)
    negative = np.empty((N_NODES, 64), np.float32)
    colsum = np.zeros(128, np.float64)
    for p in range(N_CORES):
        out = res2[p]["out"]
        positive[p * RPC:(p + 1) * RPC] = out[:RPC, :64]
        negative[p * RPC:(p + 1) * RPC] = out[:RPC, 64:]
        colsum += res2[p]["colsum"][:, 0].astype(np.float64)
    mean_pos = colsum[:64] / N_NODES
    summary = (1.0 / (1.0 + np.exp(-mean_pos))).astype(np.float32)
    return positive, negative, summary


# revision 24
# speedup vs baseline: 17286.6061x; 17286.6061x over previous
"""DGI (2-layer GCN encoder) Trainium2 kernel, 8-core SPMD.

Strategy:
- Fuse positive/negative streams as 128-wide features: row c of the launch-1
  gather table = [x[c], x[perm[c]]] * dis[c]  (512B rows).
- Shard destination nodes across 8 cores (12500 rows each, padded to 98
  blocks of 128 rows, processed in superblocks of 8 blocks). Each core
  bulk-gathers source rows from its local HBM copy of the table with
  dma_gather (int16 indices -> 4 table chunks of 32768 rows), segment-sums
  them via one-hot S-matrix matmuls accumulating into PSUM (aggT[fi, r]),
  then applies the 128x128 block-diagonal weight once per 128-row block,
  dis[r] as a per-partition scale on the PSUM->SBUF copy, bias add, PReLU.
- Between layers the host reassembles the full 100k x 128 activation table
  (launch-1 output rows are already dis[r]-scaled = exactly the layer-2
  gather table) and restages it; launch 2 repeats the pipeline with W2/b2,
  no PReLU, and emits masked per-feature column sums for the summary.
"""
import sys
import numpy as np

sys.path.insert(0, "/opt/trn_rl_repo")

import concourse.bass as bass
import concourse.bacc as bacc
import concourse.mybir as mybir
import concourse.tile as tile
from concourse.bass_utils import run_bass_kernel_spmd

N_NODES = 100000
N_CORES = 8
RPC = N_NODES // N_CORES          # 12500 destination rows per core
NB = (RPC + 127) // 128           # 98 blocks of 128 rows
BR = NB * 128                     # 12544 padded rows per core
F2 = 128                          # fused feature width (pos|neg)
SBB = 8                           # blocks per superblock
NSB = (NB + SBB - 1) // SBB       # 13 superblocks
# per-core rotated table: core p stages rows rotated by (p*RPC - ROT0) so its
# own 12544 rows sit inside chunk 0 for every core (uniform schedule).
ROT0 = 4096
CHUNK_BOUNDS = [0, 24576, 49718, 74859, 100000]   # widths < 32768
NCHUNK = len(CHUNK_BOUNDS) - 1
PRELU_A = 0.25
PAD_DST = 200.0                   # dst sentinel -> no one-hot match in S


def _build_launch(sched, final, prelu_a=PRELU_A, repeat=1):
    """One SPMD launch over NB blocks.

    sched: dict with the shared compile-time tile schedule:
      T_bc [NB, NCHUNK] tiles per (block, chunk); ni_sc [NSB][NCHUNK]
      gather sizes; NT total tiles; NIDX total index slots.
    final=False: layer 1 -> writes dis[r]*prelu(conv1) rows (layer-2 table).
    final=True:  layer 2 -> writes conv2 rows + masked column sums (summary).
    """
    T_bc, ni_sc, NT, NIDX = sched["T_bc"], sched["ni_sc"], sched["NT"], sched["NIDX"]
    nc = bacc.Bacc(num_swdge_queues=4)
    f32 = mybir.dt.float32
    d_table = nc.declare_dram_parameter("table", [N_NODES, F2], f32, isOutput=False)
    d_idx = nc.declare_dram_parameter("idx", [128, NIDX // 16], mybir.dt.int16, isOutput=False)
    d_dst = nc.declare_dram_parameter("dst", [128, NT], f32, isOutput=False)
    d_iota = nc.declare_dram_parameter("iota", [128, 128], f32, isOutput=False)
    d_w = nc.declare_dram_parameter("w", [F2, F2], f32, isOutput=False)
    d_bb = nc.declare_dram_parameter("bb", [128, 4 * F2], f32, isOutput=False)
    d_dis = nc.declare_dram_parameter("dis", [128, NB], f32, isOutput=False)
    d_out = nc.declare_dram_parameter("out", [BR, F2], f32, isOutput=True)
    if final:
        d_mask = nc.declare_dram_parameter("mask", [128, NB], f32, isOutput=False)
        d_sum = nc.declare_dram_parameter("colsum", [128, 1], f32, isOutput=True)

    GRP = 4  # blocks per epilogue group
    with tile.TileContext(nc) as tc:
        with (
            tc.tile_pool(name="const", bufs=1) as cpool,
            tc.tile_pool(name="gat", bufs=3) as gpool,
            tc.tile_pool(name="sbl", bufs=2) as spool,
            tc.tile_pool(name="agc", bufs=2) as apool,
            tc.tile_pool(name="epi", bufs=3) as epool,
            tc.tile_pool(name="pa", bufs=2, space="PSUM") as pa,
            tc.tile_pool(name="py", bufs=2, space="PSUM") as py,
            tc.tile_pool(name="ps", bufs=1, space="PSUM") as ps,
        ):
            t_idx = cpool.tile([128, NIDX // 16], mybir.dt.int16)
            t_dst = cpool.tile([128, NT], f32)
            t_iota = cpool.tile([128, 128], f32)
            t_w = cpool.tile([F2, F2], f32)
            t_bb = cpool.tile([128, 4 * F2], f32)
            t_dis = cpool.tile([128, NB], f32)
            nc.sync.dma_start(out=t_idx[:], in_=d_idx[:])
            nc.sync.dma_start(out=t_dst[:], in_=d_dst[:])
            nc.sync.dma_start(out=t_iota[:], in_=d_iota[:])
            nc.sync.dma_start(out=t_w[:], in_=d_w[:])
            nc.sync.dma_start(out=t_bb[:], in_=d_bb[:])
            nc.sync.dma_start(out=t_dis[:], in_=d_dis[:])
            if final:
                t_mask = cpool.tile([128, NB], f32)
                nc.sync.dma_start(out=t_mask[:], in_=d_mask[:])
                t_sumps = ps.tile([128, 1], f32, space="PSUM")

            def _one_pass():
              idx_off = 0   # in 16-wide slots
              tile_off = 0  # global tile counter
              for sb in range(NSB):
                blks = list(range(sb * SBB, min((sb + 1) * SBB, NB)))
                nblk = len(blks)
                aggT = pa.tile([128, SBB * 128], f32, space="PSUM", tag="aggT")
                # emission order of aggT matmuls; start/stop once per 2KB
                # PSUM zero-region (4 block-slices of 128 fp32 columns)
                seq = [(c, j, t)
                       for c in range(NCHUNK)
                       for j, b in enumerate(blks)
                       for t in range(int(T_bc[b, c]))]
                region = lambda j: j // 4
                first_of = {}
                last_of = {}
                for k, (c, j, t) in enumerate(seq):
                    r = region(j)
                    if r not in first_of:
                        first_of[r] = k
                    last_of[r] = k
                k_iter = 0
                for c in range(NCHUNK):
                    ni = ni_sc[sb][c]
                    if ni == 0:
                        continue
                    ntile = ni // 128
                    gbuf = gpool.tile([128, ntile * F2], f32, tag="gbuf")
                    nc.gpsimd.dma_gather(
                        out_ap=gbuf[:].rearrange("p (t f) -> p t f", f=F2),
                        in_ap=d_table[CHUNK_BOUNDS[c]:CHUNK_BOUNDS[c + 1], :],
                        idxs_ap=t_idx[:, idx_off:idx_off + ni // 16],
                        num_idxs=ni, num_idxs_reg=ni,
                        elem_size=F2, single_packet=False,
                        queue_num=(sb * NCHUNK + c) % 4,
                    )
                    idx_off += ni // 16
                    S = spool.tile([128, ntile * 128], f32, tag="S")
                    nc.vector.tensor_tensor(
                        out=S[:].rearrange("p (t r) -> p t r", t=ntile),
                        in0=t_dst[:, tile_off:tile_off + ntile]
                            .to_broadcast([128, ntile, 128]),
                        in1=t_iota[:, :].rearrange("p (o r) -> p o r", o=1)
                            .to_broadcast([128, ntile, 128]),
                        op=mybir.AluOpType.is_equal,
                    )
                    ti = 0
                    for j, b in enumerate(blks):
                        Tb = int(T_bc[b, c])
                        for t in range(Tb):
                            r = region(j)
                            nc.tensor.matmul(
                                out=aggT[:, j * 128:(j + 1) * 128],
                                lhsT=gbuf[:, ti * F2:(ti + 1) * F2],
                                rhs=S[:, ti * 128:(ti + 1) * 128],
                                start=(first_of[r] == k_iter),
                                stop=(last_of[r] == k_iter),
                                skip_group_check=True,
                            )
                            ti += 1
                            k_iter += 1
                    assert ti == ntile
                    tile_off += ntile
                aggTs = apool.tile([128, SBB * 128], f32, tag="aggTs")
                nc.vector.tensor_copy(out=aggTs[:, :nblk * 128],
                                      in_=aggT[:, :nblk * 128])
                for g0 in range(0, nblk, GRP):
                    gblks = blks[g0:g0 + GRP]
                    ng = len(gblks)
                    y_grp = py.tile([128, GRP * F2], f32, space="PSUM", tag="y")
                    o_grp = epool.tile([128, GRP * F2], f32, tag="o")
                    for j, b in enumerate(gblks):
                        nc.tensor.matmul(
                            out=y_grp[:, j * F2:(j + 1) * F2],
                            lhsT=aggTs[:, (g0 + j) * 128:(g0 + j + 1) * 128],
                            rhs=t_w[:], start=True, stop=True)
                        nc.scalar.activation(
                            out=o_grp[:, j * F2:(j + 1) * F2],
                            in_=y_grp[:, j * F2:(j + 1) * F2],
                            func=mybir.ActivationFunctionType.Copy,
                            scale=t_dis[:, b:b + 1])
                    w_grp = slice(0, ng * F2)
                    nc.vector.tensor_tensor(out=o_grp[:, w_grp], in0=o_grp[:, w_grp],
                                            in1=t_bb[:, w_grp], op=mybir.AluOpType.add)
                    if not final:
                        nc.vector.scalar_tensor_tensor(
                            out=o_grp[:, w_grp], in0=o_grp[:, w_grp],
                            scalar=float(prelu_a), in1=o_grp[:, w_grp],
                            op0=mybir.AluOpType.mult, op1=mybir.AluOpType.max)
                        for j, b in enumerate(gblks):
                            nc.scalar.activation(
                                out=o_grp[:, j * F2:(j + 1) * F2],
                                in_=o_grp[:, j * F2:(j + 1) * F2],
                                func=mybir.ActivationFunctionType.Copy,
                                scale=t_dis[:, b:b + 1])
                    else:
                        for j, b in enumerate(gblks):
                            nc.tensor.matmul(
                                out=t_sumps[:],
                                lhsT=o_grp[:, j * F2:(j + 1) * F2],
                                rhs=t_mask[:, b:b + 1],
                                start=(b == 0), stop=(b == NB - 1))
                    nc.sync.dma_start(
                        out=d_out[gblks[0] * 128:(gblks[0] + ng) * 128, :]
                            .rearrange("(b p) f -> p b f", p=128),
                        in_=o_grp[:, w_grp].rearrange("p (b f) -> p b f", f=F2),
                    )
            for _rep in range(repeat):
                _one_pass()
            if final:
                sums = epool.tile([128, 1], f32, tag="sums")
                nc.vector.tensor_copy(out=sums[:], in_=t_sumps[:])
                nc.sync.dma_start(out=d_sum[:], in_=sums[:])
    nc.finalize()
    return nc


def _prep_graph(edge_index):
    """Host-side index prep. Returns shared schedule + per-core arrays."""
    ei = np.asarray(edge_index)
    src = ei[0].astype(np.int64)
    dstc = ei[1].astype(np.int64)
    loops = np.arange(N_NODES, dtype=np.int64)
    rows = np.concatenate([src, loops])       # destination of each message
    cols = np.concatenate([dstc, loops])      # source of each message
    deg = np.bincount(rows, minlength=N_NODES).astype(np.float32)
    dis = (1.0 / np.sqrt(deg)).astype(np.float32)

    bounds = np.asarray(CHUNK_BOUNDS)
    cores = []
    counts_all = np.zeros((N_CORES, NB, NCHUNK), np.int64)
    for p in range(N_CORES):
        lo = p * RPC
        m = (rows >= lo) & (rows < lo + RPC)
        r_loc = rows[m] - lo
        # rotated source id: core table row = (col - (lo - ROT0)) mod N
        c_loc = (cols[m] - (lo - ROT0)) % N_NODES
        blk = r_loc >> 7
        chunk = np.searchsorted(bounds, c_loc, side="right") - 1
        order = np.lexsort((c_loc, chunk, blk))
        r_loc, c_loc, blk, chunk = (a[order] for a in (r_loc, c_loc, blk, chunk))
        key = blk * NCHUNK + chunk
        counts_all[p] = np.bincount(key, minlength=NB * NCHUNK).reshape(NB, NCHUNK)
        cores.append((r_loc, c_loc, key))

    T_bc = (counts_all.max(axis=0) + 127) >> 7       # [NB, NCHUNK] shared
    # gather sizes per (superblock, chunk)
    ni_sc = [[int(T_bc[sb * SBB:(sb + 1) * SBB, c].sum()) * 128
              for c in range(NCHUNK)] for c_ in [0] for sb in range(NSB)]
    NT = int(T_bc.sum())
    NIDX = NT * 128

    # per-(b,c) slot offsets in schedule order (sb, c, b, t)
    slot_off = np.zeros((NB, NCHUNK), np.int64)
    off = 0
    order_bc = []
    for sb in range(NSB):
        for c in range(NCHUNK):
            for b in range(sb * SBB, min((sb + 1) * SBB, NB)):
                slot_off[b, c] = off
                off += int(T_bc[b, c])
                order_bc.append((b, c))
    assert off == NT

    idx_all, dst_all, dis_all, mask_all = [], [], [], []
    for p in range(N_CORES):
        r_loc, c_loc, key = cores[p]
        starts = np.zeros(NB * NCHUNK + 1, np.int64)
        np.cumsum(np.bincount(key, minlength=NB * NCHUNK), out=starts[1:])
        idx_flat = np.zeros(NT * 128, np.int64)
        dst_flat = np.full(NT * 128, PAD_DST, np.float32)
        for b in range(NB):
            for c in range(NCHUNK):
                nE = int(counts_all[p, b, c])
                if T_bc[b, c] == 0:
                    continue
                s0 = int(slot_off[b, c]) * 128
                cap = int(T_bc[b, c]) * 128
                sl = slice(starts[b * NCHUNK + c], starts[b * NCHUNK + c] + nE)
                idx_flat[s0:s0 + nE] = c_loc[sl] - CHUNK_BOUNDS[c]
                if nE:
                    idx_flat[s0 + nE:s0 + cap] = idx_flat[s0 + nE - 1]
                dst_flat[s0:s0 + nE] = (r_loc[sl] & 127).astype(np.float32)
        # idx16: index j of the flat stream at [16*(rep) + j%16, j//16]
        idx16 = np.zeros((128, NT * 8), np.int16)
        w = idx_flat.reshape(NT * 8, 16).T.astype(np.int16)
        for rep in range(8):
            idx16[rep * 16:(rep + 1) * 16, :] = w
        # dst: tile t column, partition e = edge (t*128+e)
        dst_dev = np.ascontiguousarray(dst_flat.reshape(NT, 128).T)
        dis_pad = np.zeros(BR, np.float32)
        dis_pad[:RPC] = dis[p * RPC:(p + 1) * RPC]
        dis_dev = np.ascontiguousarray(dis_pad.reshape(NB, 128).T)
        maskv = np.zeros(BR, np.float32)
        maskv[:RPC] = 1.0
        mask_dev = np.ascontiguousarray(maskv.reshape(NB, 128).T)
        idx_all.append(idx16); dst_all.append(dst_dev)
        dis_all.append(dis_dev); mask_all.append(mask_dev)

    sched = {"T_bc": T_bc, "ni_sc": ni_sc, "NT": NT, "NIDX": NIDX}
    return sched, dis, idx_all, dst_all, dis_all, mask_all


_iota = np.tile(np.arange(128, dtype=np.float32), (128, 1))


def _launch(nc, base_maps, extra):
    in_maps = [dict(bm, **ex) for bm, ex in zip(base_maps, extra)]
    last_err = None
    for _attempt in range(3):
        try:
            return run_bass_kernel_spmd(nc, in_maps, list(range(N_CORES))).results
        except Exception as e:  # transient NRT_EXEC_UNIT_UNRECOVERABLE etc.
            last_err = e
    raise last_err


def _bd(W):
    Z = np.zeros((F2, F2), np.float32)
    Z[:64, :64] = W
    Z[64:, 64:] = W
    return Z


def kernel(x, edge_index, perm, W1, b1, prelu_a, W2, b2):
    x = np.asarray(x, np.float32)
    W1 = np.asarray(W1, np.float32); b1 = np.asarray(b1, np.float32)
    W2 = np.asarray(W2, np.float32); b2 = np.asarray(b2, np.float32)
    perm_np = np.asarray(perm).astype(np.int64)

    sched, dis, idx_all, dst_all, dis_all, mask_all = _prep_graph(edge_index)

    nc1 = _build_launch(sched, final=False, prelu_a=float(np.asarray(prelu_a)))
    nc2 = _build_launch(sched, final=True)

    table1 = np.concatenate([x, x[perm_np]], axis=1) * dis[:, None]
    bb1 = np.tile(np.concatenate([b1, b1])[None, :], (128, 4)).astype(np.float32)
    bb2 = np.tile(np.concatenate([b2, b2])[None, :], (128, 4)).astype(np.float32)

    def rot(tab, p):
        s = (p * RPC - ROT0) % N_NODES
        return np.ascontiguousarray(np.concatenate([tab[s:], tab[:s]], axis=0))

    base1 = [{"table": rot(table1, p), "iota": _iota, "w": _bd(W1), "bb": bb1}
             for p in range(N_CORES)]
    extra1 = [{"idx": idx_all[p], "dst": dst_all[p], "dis": dis_all[p]}
              for p in range(N_CORES)]
    res1 = _launch(nc1, base1, extra1)

    table2 = np.empty((N_NODES, F2), np.float32)
    for p in range(N_CORES):
        table2[p * RPC:(p + 1) * RPC] = res1[p]["out"][:RPC]

    base2 = [{"table": rot(table2, p), "iota": _iota, "w": _bd(W2), "bb": bb2}
             for p in range(N_CORES)]
    extra2 = [{"idx": idx_all[p], "dst": dst_all[p], "dis": dis_all[p],
               "mask": mask_all[p]} for p in range(N_CORES)]
    res2 = _launch(nc2, base2, extra2)

    positive = np.empty((N_NODES, 64), np.float32)
    negative = np.empty((N_NODES, 64), np.float32)
    colsum = np.zeros(128, np.float64)
    for p in range(N_CORES):
        out = res2[p]["out"]
        positive[p * RPC:(p + 1) * RPC] = out[:RPC, :64]
        negative[p * RPC:(p + 1) * RPC] = out[:RPC, 64:]
        colsum += res2[p]["colsum"][:, 0].astype(np.float64)
    mean_pos = colsum[:64] / N_NODES
    summary = (1.0 / (1.0 + np.exp(-mean_pos))).astype(np.float32)
    return positive, negative, summary


# revision 28
# speedup vs baseline: 148356.4038x; 8.5822x over previous
"""DGI (2-layer GCN encoder) Trainium2 kernel, 8-core SPMD.

Strategy:
- Fuse positive/negative streams as 128-wide features: row c of the launch-1
  gather table = [x[c], x[perm[c]]] * dis[c]  (512B rows).
- Shard destination nodes across 8 cores (12500 rows each, padded to 98
  blocks of 128 rows, processed in superblocks of 8 blocks). Each core
  bulk-gathers source rows from its local HBM copy of the table with
  dma_gather (int16 indices -> 4 table chunks of 32768 rows), segment-sums
  them via one-hot S-matrix matmuls accumulating into PSUM (aggT[fi, r]),
  then applies the 128x128 block-diagonal weight once per 128-row block,
  dis[r] as a per-partition scale on the PSUM->SBUF copy, bias add, PReLU.
- Between layers the host reassembles the full 100k x 128 activation table
  (launch-1 output rows are already dis[r]-scaled = exactly the layer-2
  gather table) and restages it; launch 2 repeats the pipeline with W2/b2,
  no PReLU, and emits masked per-feature column sums for the summary.
"""
import sys
import numpy as np

sys.path.insert(0, "/opt/trn_rl_repo")

import concourse.bass as bass
import concourse.bacc as bacc
import concourse.mybir as mybir
import concourse.tile as tile
from concourse.bass_utils import run_bass_kernel_spmd

N_NODES = 100000
N_CORES = 8
RPC = N_NODES // N_CORES          # 12500 destination rows per core
NB = (RPC + 127) // 128           # 98 blocks of 128 rows
BR = NB * 128                     # 12544 padded rows per core
F2 = 128                          # fused feature width (pos|neg)
SBB = 8                           # blocks per superblock
NSB = (NB + SBB - 1) // SBB       # 13 superblocks
# per-core rotated table: core p stages rows rotated by (p*RPC - ROT0) so its
# own 12544 rows sit inside chunk 0 for every core (uniform schedule).
ROT0 = 4096
CHUNK_BOUNDS = [0, 24576, 49718, 74859, 100000]   # widths < 32768
NCHUNK = len(CHUNK_BOUNDS) - 1
PRELU_A = 0.25
PAD_DST = 200.0                   # dst sentinel -> no one-hot match in S


def _build_launch(sched, final, prelu_a=PRELU_A, repeat=1, bufs_g=6, bufs_s=3,
                  qsplit=2):
    """One SPMD launch over NB blocks.

    sched: dict with the shared compile-time tile schedule:
      T_bc [NB, NCHUNK] tiles per (block, chunk); ni_sc [NSB][NCHUNK]
      gather sizes; NT total tiles; NIDX total index slots.
    final=False: layer 1 -> writes dis[r]*prelu(conv1) rows (layer-2 table).
    final=True:  layer 2 -> writes conv2 rows + masked column sums (summary).
    """
    T_bc, ni_sc, NT, NIDX = sched["T_bc"], sched["ni_sc"], sched["NT"], sched["NIDX"]
    nc = bacc.Bacc(num_swdge_queues=4)
    f32 = mybir.dt.float32
    d_table = nc.declare_dram_parameter("table", [N_NODES, F2], f32, isOutput=False)
    d_idx = nc.declare_dram_parameter("idx", [128, NIDX // 16], mybir.dt.int16, isOutput=False)
    d_dst = nc.declare_dram_parameter("dst", [128, NT], f32, isOutput=False)
    d_iota = nc.declare_dram_parameter("iota", [128, 128], f32, isOutput=False)
    d_w = nc.declare_dram_parameter("w", [F2, F2], f32, isOutput=False)
    d_bb = nc.declare_dram_parameter("bb", [128, 4 * F2], f32, isOutput=False)
    d_dis = nc.declare_dram_parameter("dis", [128, NB], f32, isOutput=False)
    d_out = nc.declare_dram_parameter("out", [BR, F2], f32, isOutput=True)
    if final:
        d_mask = nc.declare_dram_parameter("mask", [128, NB], f32, isOutput=False)
        d_sum = nc.declare_dram_parameter("colsum", [128, 1], f32, isOutput=True)

    GRP = 4  # blocks per epilogue group
    with tile.TileContext(nc) as tc:
        with (
            tc.tile_pool(name="const", bufs=1) as cpool,
            tc.tile_pool(name="gat", bufs=bufs_g) as gpool,
            tc.tile_pool(name="sbl", bufs=bufs_s) as spool,
            tc.tile_pool(name="agc", bufs=2) as apool,
            tc.tile_pool(name="epi", bufs=3) as epool,
            tc.tile_pool(name="pa", bufs=2, space="PSUM") as pa,
            tc.tile_pool(name="py", bufs=2, space="PSUM") as py,
            tc.tile_pool(name="ps", bufs=1, space="PSUM") as ps,
        ):
            t_idx = cpool.tile([128, NIDX // 16], mybir.dt.int16)
            t_dst = cpool.tile([128, NT], f32)
            t_iota = cpool.tile([128, 128], f32)
            t_w = cpool.tile([F2, F2], f32)
            t_bb = cpool.tile([128, 4 * F2], f32)
            t_dis = cpool.tile([128, NB], f32)
            nc.sync.dma_start(out=t_idx[:], in_=d_idx[:])
            nc.sync.dma_start(out=t_dst[:], in_=d_dst[:])
            nc.sync.dma_start(out=t_iota[:], in_=d_iota[:])
            nc.sync.dma_start(out=t_w[:], in_=d_w[:])
            nc.sync.dma_start(out=t_bb[:], in_=d_bb[:])
            nc.sync.dma_start(out=t_dis[:], in_=d_dis[:])
            if final:
                t_mask = cpool.tile([128, NB], f32)
                nc.sync.dma_start(out=t_mask[:], in_=d_mask[:])
                t_sumps = ps.tile([128, 1], f32, space="PSUM")

            def _one_pass():
              idx_off = 0   # in 16-wide slots
              tile_off = 0  # global tile counter
              for sb in range(NSB):
                blks = list(range(sb * SBB, min((sb + 1) * SBB, NB)))
                nblk = len(blks)
                aggT = pa.tile([128, SBB * 128], f32, space="PSUM", tag="aggT")
                # emission order of aggT matmuls; start/stop once per 2KB
                # PSUM zero-region (4 block-slices of 128 fp32 columns)
                seq = [(c, j, t)
                       for c in range(NCHUNK)
                       for j, b in enumerate(blks)
                       for t in range(int(T_bc[b, c]))]
                region = lambda j: j // 4
                first_of = {}
                last_of = {}
                for k, (c, j, t) in enumerate(seq):
                    r = region(j)
                    if r not in first_of:
                        first_of[r] = k
                    last_of[r] = k
                k_iter = 0
                for c in range(NCHUNK):
                    ni = ni_sc[sb][c]
                    if ni == 0:
                        continue
                    ntile = ni // 128
                    gbuf = gpool.tile([128, ntile * F2], f32, tag="gbuf")
                    # split the gather into qsplit pieces on distinct queues
                    # so several SDMA queue rows drain concurrently
                    nsplit = max(1, min(qsplit, ntile))
                    t0s = [(ntile * s) // nsplit for s in range(nsplit + 1)]
                    for s in range(nsplit):
                        tl, th = t0s[s], t0s[s + 1]
                        if th == tl:
                            continue
                        nis = (th - tl) * 128
                        nc.gpsimd.dma_gather(
                            out_ap=gbuf[:, tl * F2:th * F2]
                                .rearrange("p (t f) -> p t f", f=F2),
                            in_ap=d_table[CHUNK_BOUNDS[c]:CHUNK_BOUNDS[c + 1], :],
                            idxs_ap=t_idx[:, idx_off + tl * 8:idx_off + th * 8],
                            num_idxs=nis, num_idxs_reg=nis,
                            elem_size=F2, single_packet=False,
                            queue_num=(sb * NCHUNK * nsplit + c * nsplit + s) % 4,
                        )
                    idx_off += ni // 16
                    S = spool.tile([128, ntile * 128], f32, tag="S")
                    nc.vector.tensor_tensor(
                        out=S[:].rearrange("p (t r) -> p t r", t=ntile),
                        in0=t_dst[:, tile_off:tile_off + ntile]
                            .to_broadcast([128, ntile, 128]),
                        in1=t_iota[:, :].rearrange("p (o r) -> p o r", o=1)
                            .to_broadcast([128, ntile, 128]),
                        op=mybir.AluOpType.is_equal,
                    )
                    ti = 0
                    for j, b in enumerate(blks):
                        Tb = int(T_bc[b, c])
                        for t in range(Tb):
                            r = region(j)
                            nc.tensor.matmul(
                                out=aggT[:, j * 128:(j + 1) * 128],
                                lhsT=gbuf[:, ti * F2:(ti + 1) * F2],
                                rhs=S[:, ti * 128:(ti + 1) * 128],
                                start=(first_of[r] == k_iter),
                                stop=(last_of[r] == k_iter),
                                skip_group_check=True,
                            )
                            ti += 1
                            k_iter += 1
                    assert ti == ntile
                    tile_off += ntile
                aggTs = apool.tile([128, SBB * 128], f32, tag="aggTs")
                nc.vector.tensor_copy(out=aggTs[:, :nblk * 128],
                                      in_=aggT[:, :nblk * 128])
                for g0 in range(0, nblk, GRP):
                    gblks = blks[g0:g0 + GRP]
                    ng = len(gblks)
                    y_grp = py.tile([128, GRP * F2], f32, space="PSUM", tag="y")
                    o_grp = epool.tile([128, GRP * F2], f32, tag="o")
                    for j, b in enumerate(gblks):
                        nc.tensor.matmul(
                            out=y_grp[:, j * F2:(j + 1) * F2],
                            lhsT=aggTs[:, (g0 + j) * 128:(g0 + j + 1) * 128],
                            rhs=t_w[:], start=True, stop=True)
                        nc.scalar.activation(
                            out=o_grp[:, j * F2:(j + 1) * F2],
                            in_=y_grp[:, j * F2:(j + 1) * F2],
                            func=mybir.ActivationFunctionType.Copy,
                            scale=t_dis[:, b:b + 1])
                    w_grp = slice(0, ng * F2)
                    nc.vector.tensor_tensor(out=o_grp[:, w_grp], in0=o_grp[:, w_grp],
                                            in1=t_bb[:, w_grp], op=mybir.AluOpType.add)
                    if not final:
                        nc.vector.scalar_tensor_tensor(
                            out=o_grp[:, w_grp], in0=o_grp[:, w_grp],
                            scalar=float(prelu_a), in1=o_grp[:, w_grp],
                            op0=mybir.AluOpType.mult, op1=mybir.AluOpType.max)
                        for j, b in enumerate(gblks):
                            nc.scalar.activation(
                                out=o_grp[:, j * F2:(j + 1) * F2],
                                in_=o_grp[:, j * F2:(j + 1) * F2],
                                func=mybir.ActivationFunctionType.Copy,
                                scale=t_dis[:, b:b + 1])
                    else:
                        for j, b in enumerate(gblks):
                            nc.tensor.matmul(
                                out=t_sumps[:],
                                lhsT=o_grp[:, j * F2:(j + 1) * F2],
                                rhs=t_mask[:, b:b + 1],
                                start=(b == 0), stop=(b == NB - 1))
                    nc.sync.dma_start(
                        out=d_out[gblks[0] * 128:(gblks[0] + ng) * 128, :]
                            .rearrange("(b p) f -> p b f", p=128),
                        in_=o_grp[:, w_grp].rearrange("p (b f) -> p b f", f=F2),
                    )
            for _rep in range(repeat):
                _one_pass()
            if final:
                sums = epool.tile([128, 1], f32, tag="sums")
                nc.vector.tensor_copy(out=sums[:], in_=t_sumps[:])
                nc.sync.dma_start(out=d_sum[:], in_=sums[:])
    nc.finalize()
    return nc


def _prep_graph(edge_index):
    """Host-side index prep. Returns shared schedule + per-core arrays."""
    ei = np.asarray(edge_index)
    src = ei[0].astype(np.int64)
    dstc = ei[1].astype(np.int64)
    loops = np.arange(N_NODES, dtype=np.int64)
    rows = np.concatenate([src, loops])       # destination of each message
    cols = np.concatenate([dstc, loops])      # source of each message
    deg = np.bincount(rows, minlength=N_NODES).astype(np.float32)
    dis = (1.0 / np.sqrt(deg)).astype(np.float32)

    bounds = np.asarray(CHUNK_BOUNDS)
    cores = []
    counts_all = np.zeros((N_CORES, NB, NCHUNK), np.int64)
    for p in range(N_CORES):
        lo = p * RPC
        m = (rows >= lo) & (rows < lo + RPC)
        r_loc = rows[m] - lo
        # rotated source id: core table row = (col - (lo - ROT0)) mod N
        c_loc = (cols[m] - (lo - ROT0)) % N_NODES
        blk = r_loc >> 7
        chunk = np.searchsorted(bounds, c_loc, side="right") - 1
        order = np.lexsort((c_loc, chunk, blk))
        r_loc, c_loc, blk, chunk = (a[order] for a in (r_loc, c_loc, blk, chunk))
        key = blk * NCHUNK + chunk
        counts_all[p] = np.bincount(key, minlength=NB * NCHUNK).reshape(NB, NCHUNK)
        cores.append((r_loc, c_loc, key))

    T_bc = (counts_all.max(axis=0) + 127) >> 7       # [NB, NCHUNK] shared
    # gather sizes per (superblock, chunk)
    ni_sc = [[int(T_bc[sb * SBB:(sb + 1) * SBB, c].sum()) * 128
              for c in range(NCHUNK)] for c_ in [0] for sb in range(NSB)]
    NT = int(T_bc.sum())
    NIDX = NT * 128

    # per-(b,c) slot offsets in schedule order (sb, c, b, t)
    slot_off = np.zeros((NB, NCHUNK), np.int64)
    off = 0
    order_bc = []
    for sb in range(NSB):
        for c in range(NCHUNK):
            for b in range(sb * SBB, min((sb + 1) * SBB, NB)):
                slot_off[b, c] = off
                off += int(T_bc[b, c])
                order_bc.append((b, c))
    assert off == NT

    idx_all, dst_all, dis_all, mask_all = [], [], [], []
    for p in range(N_CORES):
        r_loc, c_loc, key = cores[p]
        starts = np.zeros(NB * NCHUNK + 1, np.int64)
        np.cumsum(np.bincount(key, minlength=NB * NCHUNK), out=starts[1:])
        idx_flat = np.zeros(NT * 128, np.int64)
        dst_flat = np.full(NT * 128, PAD_DST, np.float32)
        for b in range(NB):
            for c in range(NCHUNK):
                nE = int(counts_all[p, b, c])
                if T_bc[b, c] == 0:
                    continue
                s0 = int(slot_off[b, c]) * 128
                cap = int(T_bc[b, c]) * 128
                sl = slice(starts[b * NCHUNK + c], starts[b * NCHUNK + c] + nE)
                idx_flat[s0:s0 + nE] = c_loc[sl] - CHUNK_BOUNDS[c]
                if nE:
                    idx_flat[s0 + nE:s0 + cap] = idx_flat[s0 + nE - 1]
                dst_flat[s0:s0 + nE] = (r_loc[sl] & 127).astype(np.float32)
        # idx16: index j of the flat stream at [16*(rep) + j%16, j//16]
        idx16 = np.zeros((128, NT * 8), np.int16)
        w = idx_flat.reshape(NT * 8, 16).T.astype(np.int16)
        for rep in range(8):
            idx16[rep * 16:(rep + 1) * 16, :] = w
        # dst: tile t column, partition e = edge (t*128+e)
        dst_dev = np.ascontiguousarray(dst_flat.reshape(NT, 128).T)
        dis_pad = np.zeros(BR, np.float32)
        dis_pad[:RPC] = dis[p * RPC:(p + 1) * RPC]
        dis_dev = np.ascontiguousarray(dis_pad.reshape(NB, 128).T)
        maskv = np.zeros(BR, np.float32)
        maskv[:RPC] = 1.0
        mask_dev = np.ascontiguousarray(maskv.reshape(NB, 128).T)
        idx_all.append(idx16); dst_all.append(dst_dev)
        dis_all.append(dis_dev); mask_all.append(mask_dev)

    sched = {"T_bc": T_bc, "ni_sc": ni_sc, "NT": NT, "NIDX": NIDX}
    return sched, dis, idx_all, dst_all, dis_all, mask_all


_iota = np.tile(np.arange(128, dtype=np.float32), (128, 1))


def _launch(nc, base_maps, extra):
    in_maps = [dict(bm, **ex) for bm, ex in zip(base_maps, extra)]
    last_err = None
    for _attempt in range(3):
        try:
            return run_bass_kernel_spmd(nc, in_maps, list(range(N_CORES))).results
        except Exception as e:  # transient NRT_EXEC_UNIT_UNRECOVERABLE etc.
            last_err = e
    raise last_err


def _bd(W):
    Z = np.zeros((F2, F2), np.float32)
    Z[:64, :64] = W
    Z[64:, 64:] = W
    return Z


def kernel(x, edge_index, perm, W1, b1, prelu_a, W2, b2):
    x = np.asarray(x, np.float32)
    W1 = np.asarray(W1, np.float32); b1 = np.asarray(b1, np.float32)
    W2 = np.asarray(W2, np.float32); b2 = np.asarray(b2, np.float32)
    perm_np = np.asarray(perm).astype(np.int64)

    sched, dis, idx_all, dst_all, dis_all, mask_all = _prep_graph(edge_index)

    nc1 = _build_launch(sched, final=False, prelu_a=float(np.asarray(prelu_a)))
    nc2 = _build_launch(sched, final=True)

    table1 = np.concatenate([x, x[perm_np]], axis=1) * dis[:, None]
    bb1 = np.tile(np.concatenate([b1, b1])[None, :], (128, 4)).astype(np.float32)
    bb2 = np.tile(np.concatenate([b2, b2])[None, :], (128, 4)).astype(np.float32)

    def rot(tab, p):
        s = (p * RPC - ROT0) % N_NODES
        return np.ascontiguousarray(np.concatenate([tab[s:], tab[:s]], axis=0))

    base1 = [{"table": rot(table1, p), "iota": _iota, "w": _bd(W1), "bb": bb1}
             for p in range(N_CORES)]
    extra1 = [{"idx": idx_all[p], "dst": dst_all[p], "dis": dis_all[p]}
              for p in range(N_CORES)]
    res1 = _launch(nc1, base1, extra1)

    table2 = np.empty((N_NODES, F2), np.float32)
    for p in range(N_CORES):
        table2[p * RPC:(p + 1) * RPC] = res1[p]["out"][:RPC]

    base2 = [{"table": rot(table2, p), "iota": _iota, "w": _bd(W2), "bb": bb2}
             for p in range(N_CORES)]
    extra2 = [{"idx": idx_all[p], "dst": dst_all[p], "dis": dis_all[p],
               "mask": mask_all[p]} for p in range(N_CORES)]
    res2 = _launch(nc2, base2, extra2)

    positive = np.empty((N_NODES, 64), np.float32)
    negative = np.empty((N_NODES, 64), np.float32)
    colsum = np.zeros(128, np.float64)
    for p in range(N_CORES):
        out = res2[p]["out"]
        positive[p * RPC:(p + 1) * RPC] = out[:RPC, :64]
        negative[p * RPC:(p + 1) * RPC] = out[:RPC, 64:]
        colsum += res2[p]["colsum"][:, 0].astype(np.float64)
    mean_pos = colsum[:64] / N_NODES
    summary = (1.0 / (1.0 + np.exp(-mean_pos))).astype(np.float32)
    return positive, negative, summary
